# revision 9
# baseline (speedup 1.0000x reference)
"""Batched-normalize softmax pipeline (v3).

Per chunk a (16 head-pairs, TCH=512 tokens):
  S0 (iter a):   QKV projections (PE) -> qk/v tiles
  S1 (iter a+1): per pair: scores mm (PE, hh-major 2-bank psum so the two
                 concurrent tile_position matmuls hit different banks) ->
                 exp (ACT) -> po mm (PE; per-head 128-col v block with a
                 leading ones column puts the softmax denominator in po row
                 0 and v outputs in rows 64-127) -> po_sb copy (ACT, bf16)
                 + denominator reciprocal_approx_fast (DVE, f32, shift-free
                 partition-0 op)
                 per 2 pairs: ACT f32->bf16 convert + ONE gpsimd
                 partition_broadcast (amortizes the Q7 launch; keeps the
                 broadcast off the per-pair critical path)
  S2 (iter a+2): per pair muls (DVE, all-SBUF bf16) -> on tiles
  S3 (iter a+2): out-projection (PE) -> y DMA
PSUM: pex 2x2 banks (scores) + pall 2 (proj/out psums) + ppo 2 (po) = 8.

HW lessons encoded here:
  - per-pair gpsimd ops serialize (~3us launch each on HW vs 95ns in the
    cost model); batch them.
  - nc.vector.reciprocal on a single partition is ~1-2us on HW; use
    reciprocal_approx_fast (f32-only), which also must run WITHOUT a
    partition shift (in/out both partition 0) to produce correct results.
  - SBUF APs must start at partition 0/32/64/96 and may not span more
    than the alignment gap (start 32 -> max 32 partitions).
"""
import numpy as np

import concourse.bass as bass
import concourse.mybir as mybir
import concourse.tile as tile
from concourse import bacc

P = 128
B, T, C = 4, 4096, 1024
H = 16
HD = 64
BS = 256
NB_TOTAL = (B * T) // BS
N_CORES = 8
NB = NB_TOTAL // N_CORES
TOK = NB * BS
KT = C // P
NPAIR = H // 2
TCH = 512
NCH = TOK // TCH

f32 = mybir.dt.float32
bf16 = mybir.dt.bfloat16
W_DT = bf16
ATT_DT = bf16

PAIRS = [(bl, p_) for bl in range(2) for p_ in range(NPAIR)]


def _build(reps: int = 1, variant: str = 'full'):
    nc = bacc.Bacc(None)

    xT = nc.dram_tensor("xT", [P, KT * NCH * TCH], W_DT, kind="ExternalInput")
    wqk = nc.dram_tensor("wqk", [P, 16 * KT * P], W_DT, kind="ExternalInput")
    wv = nc.dram_tensor("wv", [P, KT * C], W_DT, kind="ExternalInput")
    wout = nc.dram_tensor("wout", [P, KT * 8 * P], W_DT, kind="ExternalInput")
    bqk = nc.dram_tensor("bqk", [P, 16], f32, kind="ExternalInput")
    bout = nc.dram_tensor("bout", [P, 8], f32, kind="ExternalInput")
    yT = nc.dram_tensor("yT", [P, 8 * NCH * TCH], f32, kind="ExternalOutput")

    with tile.TileContext(nc) as tc:
        with (
            tc.tile_pool(name="wpool", bufs=1) as wpool,
            tc.tile_pool(name="xpool", bufs=2) as xpool,
            tc.tile_pool(name="qkpool", bufs=32) as qkpool,
            tc.tile_pool(name="vpool", bufs=8) as vpool,
            tc.tile_pool(name="epool", bufs=2) as epool,
            tc.tile_pool(name="popool", bufs=16) as popool,
            tc.tile_pool(name="rcppool", bufs=2) as rcppool,
            tc.tile_pool(name="rcrpool", bufs=8) as rcrpool,
            tc.tile_pool(name="onpool", bufs=8) as onpool,
            tc.tile_pool(name="ypool", bufs=2) as ypool,
            tc.tile_pool(name="pall", bufs=2, space="PSUM") as pall,
            tc.tile_pool(name="pex", bufs=2, space="PSUM") as pex,
            tc.tile_pool(name="ppo", bufs=2, space="PSUM") as ppo,
        ):
            xT_r = xT[:].rearrange("p (k c n) -> p k c n", k=KT, c=NCH)

            bqk_t = wpool.tile([P, 16], f32)
            nc.sync.dma_start(out=bqk_t[:], in_=bqk[:])
            bout_t = wpool.tile([P, 8], f32)
            nc.sync.dma_start(out=bout_t[:], in_=bout[:])
            ones_f = wpool.tile([P, 16], f32)
            nc.vector.memset(ones_f[:], 1.0)
            ones16 = wpool.tile([P, 16], ATT_DT)
            nc.vector.tensor_copy(ones16[:], ones_f[:])

            def fetch_x(c):
                xt = xpool.tile([P, KT * TCH], W_DT, tag="x")
                nc.sync.dma_start(
                    out=xt[:].rearrange("p (k n) -> p k n", k=KT),
                    in_=xT_r[:, :, c, :])
                return xt

            # single-shot path: the DMA queue drains in emission order, so
            # fetch chunk 0 of x BEFORE the 8MB of weights - projections for
            # chunk 0 then start ~5us in instead of ~30us
            x0_pre = fetch_x(0) if reps == 1 else None

            # one tile per m-slice: qk unit m only waits for its own slice
            wqk_ts = []
            for m in range(16):
                wt = wpool.tile([P, KT * P], W_DT, name=f"wqk_{m}")
                nc.sync.dma_start(
                    out=wt[:], in_=wqk[:, m * KT * P:(m + 1) * KT * P])
                wqk_ts.append(wt)
            wv_t = wpool.tile([P, KT * C], W_DT)
            for k in range(KT):
                nc.sync.dma_start(out=wv_t[:, k * C:(k + 1) * C],
                                  in_=wv[:, k * C:(k + 1) * C])
            wout_t = wpool.tile([P, KT * 8 * P], W_DT)
            for k in range(KT):
                nc.sync.dma_start(out=wout_t[:, k * 8 * P:(k + 1) * 8 * P],
                                  in_=wout[:, k * 8 * P:(k + 1) * 8 * P])

            def emit_qk_unit(xt, m, qk):
                pt = pall.tile([P, TCH], f32, tag="ps")
                for k in range(KT):
                    nc.tensor.matmul(
                        pt[:], wqk_ts[m][:, k * P:(k + 1) * P],
                        xt[:, k * TCH:(k + 1) * TCH],
                        start=(k == 0), stop=(k == KT - 1))
                st = qkpool.tile([P, TCH], ATT_DT, tag="qk")
                nc.scalar.activation(st[:], pt[:],
                                     mybir.ActivationFunctionType.Identity,
                                     bias=bqk_t[:, m:m + 1])
                qk.append(st)

            def emit_v_unit(xt, ts, vt):
                # per-head 128-col block: [ones | 63 pad | 64 v-channels].
                # po then has denom at row 0 (shift-free reciprocal) and v
                # outputs at rows 64-127 (the BIR verifier only allows SBUF
                # accesses spanning 64 partitions from start 0 or 64).
                v_sb = vpool.tile([P, 16 * 128], ATT_DT, tag="v")
                for dch in range(2):
                    pt = pall.tile([P, 512], f32, tag="ps")
                    for k in range(KT):
                        nc.tensor.matmul(
                            pt[:],
                            xt[:, k * TCH + ts * P: k * TCH + (ts + 1) * P],
                            wv_t[:, k * C + dch * 512: k * C + (dch + 1) * 512],
                            start=(k == 0), stop=(k == KT - 1))
                    nc.vector.tensor_copy(
                        v_sb[:, dch * 8 * 128:(dch + 1) * 8 * 128]
                        .rearrange("p (h cc) -> p h cc", h=8)[:, :, 64:128],
                        pt[:].rearrange("p (h cc) -> p h cc", h=8))
                nc.vector.tensor_copy(
                    v_sb[:].rearrange("p (h cc) -> p h cc", h=16)[:, :, 0:1],
                    ones16[:].unsqueeze(2))
                vt.append(v_sb)

            def emit_scores_exp(qk, bl, p_, state):
                # hh-major psum layout: the two concurrent tile_position
                # matmuls (rows 0-63 / 64-127) MUST land in different PSUM
                # banks - same-bank concurrent writes fail on HW.
                co = bl * BS
                qt, kt_ = qk[p_], qk[8 + p_]
                px = pex.tile([P, 4 * BS], f32, tag="pex")
                for jt in range(2):
                    for hh in range(2):
                        lo, hi = hh * HD, (hh + 1) * HD
                        nc.tensor.matmul(
                            px[:, (2 * hh + jt) * BS:(2 * hh + jt + 1) * BS],
                            kt_[lo:hi, co + jt * P: co + (jt + 1) * P],
                            qt[lo:hi, co:co + BS], start=True, stop=True,
                            tile_position=(hh * HD, 0))
                e = epool.tile([P, 4 * BS], ATT_DT, tag="e")
                nc.scalar.activation(
                    e[:], px[:], mybir.ActivationFunctionType.Exp)
                state["e"] = e

            def emit_po(vt, a, i, state, rcp_tiles):
                bl, p_ = PAIRS[i]
                e = state["e"]
                po = ppo.tile([P, 2 * BS], f32, tag="po")
                for hh in range(2):
                    h = 2 * p_ + hh
                    for jt in range(2):
                        nc.tensor.matmul(
                            po[:, hh * BS:(hh + 1) * BS],
                            vt[2 * bl + jt][:, h * 128:(h + 1) * 128],
                            e[:, (2 * hh + jt) * BS:(2 * hh + jt + 1) * BS],
                            start=(jt == 0), stop=(jt == 1))
                po_sb = popool.tile([P, 2 * BS], ATT_DT, tag="posb",
                                    name=f"posb_{a}_{i}")
                nc.scalar.activation(po_sb[:], po[:],
                                     mybir.ActivationFunctionType.Identity)
                if variant != 'dbg_norcp':
                    # denominators are sums of 256 exp() terms of modest
                    # scores (>~1), far from approx_fast's undefined edges
                    eighth, pp = i // 2, i % 2
                    nc.vector.reciprocal_approx_fast(
                        rcp_tiles[eighth][0:1, pp * 2 * BS:(pp + 1) * 2 * BS],
                        po[0:1, :])
                state["posb"] = po_sb

            def emit_bcast(a, quarter, rcp_t):
                rcpb = rcppool.tile([1, 2 * 2 * BS], ATT_DT, tag="rcpb",
                                    name=f"rcpb_{a}_{quarter}")
                nc.scalar.activation(rcpb[:], rcp_t[:],
                                     mybir.ActivationFunctionType.Identity)
                rcr = rcrpool.tile([P, 2 * 2 * BS], ATT_DT, tag="rcr",
                                   name=f"rcr_{a}_{quarter}")
                if variant == 'dbg_nobcast':
                    nc.vector.memset(rcr[:], 0.5)
                elif variant == 'dbg_norcp':
                    nc.vector.memset(rcr[:], 0.5)
                else:
                    nc.gpsimd.partition_broadcast(rcr[:], rcpb[:])
                return rcr

            def emit_muls(i, state, rcr, on_tiles):
                bl, p_ = PAIRS[i]
                co = bl * BS
                pp = i % 2
                po_sb = state["posb"]
                for hh in range(2):
                    nc.vector.tensor_mul(
                        on_tiles[p_][hh * HD:(hh + 1) * HD, co:co + BS],
                        po_sb[64:P, hh * BS:(hh + 1) * BS],
                        rcr[64:P,
                            pp * 2 * BS + hh * BS: pp * 2 * BS + (hh + 1) * BS])

            def emit_out_unit(on_tiles, c, t):
                pt = pall.tile([P, TCH], f32, tag="ps")
                for kk in range(KT):
                    nc.tensor.matmul(
                        pt[:], wout_t[:, (kk * 8 + t) * P:(kk * 8 + t + 1) * P],
                        on_tiles[kk][:], start=(kk == 0), stop=(kk == KT - 1))
                yt = ypool.tile([P, TCH], f32, tag="y")
                nc.scalar.activation(yt[:], pt[:],
                                     mybir.ActivationFunctionType.Identity,
                                     bias=bout_t[:, t:t + 1])
                nc.sync.dma_start(
                    out=yT[:, (t * NCH + c) * TCH:(t * NCH + c + 1) * TCH],
                    in_=yt[:])

            def body(x0=None):
                qk, vt, xts = {}, {}, {}
                states, rcps, rcrs, on = {}, {}, {}, {}

                xts[0] = x0 if x0 is not None else fetch_x(0)
                for s in range(NCH + 2):
                    A = s - 1
                    O = s - 2
                    units = []
                    if s < NCH:
                        if s + 1 < NCH:
                            units.append(lambda s=s: xts.__setitem__(
                                s + 1, fetch_x(s + 1)))
                        qk[s] = []
                        vt[s] = []
                        for m in range(16):
                            units.append(
                                lambda m=m, s=s: emit_qk_unit(xts[s], m, qk[s]))
                        for ts in range(4):
                            units.append(
                                lambda ts=ts, s=s: emit_v_unit(xts[s], ts, vt[s]))
                    late_units = []
                    if 0 <= O < NCH:
                        on[O] = [onpool.tile([P, TCH], ATT_DT, tag="on",
                                             name=f"on_{O}_{kk}")
                                 for kk in range(NPAIR)]
                        for i in range(len(PAIRS)):
                            emit_muls(i, states[O][i], rcrs[O][i // 2], on[O])
                        # out units wait on the muls -> bcast(q7) chain that
                        # only resolves a few us into this iteration; emitting
                        # them early head-of-line blocks the in-order PE queue
                        for t in range(8):
                            late_units.append(
                                lambda t=t, O=O: emit_out_unit(on[O], O, t))
                    if 0 <= A < NCH:
                        states[A] = [dict() for _ in PAIRS]
                        rcps[A] = [
                            rcppool.tile([1, 2 * 2 * BS], f32, tag="rcp",
                                         name=f"rcp_{A}_{q}")
                            for q in range(8)]
                        rcrs[A] = [None] * 8
                        ui = 0
                        prev = None
                        np_ = len(PAIRS)
                        half = np_ // 2
                        li = 0
                        for i in range(np_):
                            bl, p_ = PAIRS[i]
                            emit_scores_exp(qk[A], bl, p_, states[A][i])
                            want = (len(units) * (i + 1) + np_ - 1) // np_
                            while ui < min(len(units), want):
                                units[ui]()
                                ui += 1
                            if i >= half:
                                lwant = (len(late_units) * (i - half + 1)
                                         + half - 1) // half
                                while li < min(len(late_units), lwant):
                                    late_units[li]()
                                    li += 1
                            if prev is not None:
                                emit_po(vt[A], A, prev, states[A][prev],
                                        rcps[A])
                                if prev % 2 == 1:
                                    q = prev // 2
                                    rcrs[A][q] = emit_bcast(A, q, rcps[A][q])
                            prev = i
                        emit_po(vt[A], A, prev, states[A][prev], rcps[A])
                        rcrs[A][7] = emit_bcast(A, 7, rcps[A][7])
                        while ui < len(units):
                            units[ui]()
                            ui += 1
                        while li < len(late_units):
                            late_units[li]()
                            li += 1
                    else:
                        for u in units:
                            u()
                        for u in late_units:
                            u()
                    if 0 <= O < NCH:
                        del states[O], rcps[O], rcrs[O], on[O], qk[O], vt[O]
                        if O in xts:
                            del xts[O]

            if reps == 1:
                body(x0_pre)
            else:
                with tc.For_i(0, reps, 1):
                    body()
    nc.finalize()
    return nc


def prep_inputs(x, Wqkv, bqkv, Wout, bout):
    """Host-side shard + repack. Returns list of 8 per-core input dicts."""
    x = np.asarray(x, dtype=np.float32)
    Wqkv = np.asarray(Wqkv, dtype=np.float32)
    bqkv = np.asarray(bqkv, dtype=np.float32)
    Wout = np.asarray(Wout, dtype=np.float32)
    bout = np.asarray(bout, dtype=np.float32)
    wdt = mybir.dt.np(W_DT)

    scale = 1.0 / np.sqrt(HD)
    W3 = Wqkv.reshape(C, H, 3 * HD)
    b3 = bqkv.reshape(H, 3 * HD)
    Wq = W3[:, :, 0:HD] * scale          # [C, H, 64]
    Wk = W3[:, :, HD:2 * HD]
    Wv = W3[:, :, 2 * HD:3 * HD]
    bq = b3[:, 0:HD] * scale
    bk = b3[:, HD:2 * HD]
    bv = b3[:, 2 * HD:3 * HD]

    # m-tiles: m<8 -> [Wq_{2m} | Wq_{2m+1}], m>=8 -> k-pairs
    mt = np.empty((C, 16, P), dtype=np.float32)
    for m in range(8):
        mt[:, m, 0:HD] = Wq[:, 2 * m]
        mt[:, m, HD:P] = Wq[:, 2 * m + 1]
        mt[:, 8 + m, 0:HD] = Wk[:, 2 * m]
        mt[:, 8 + m, HD:P] = Wk[:, 2 * m + 1]
    # -> [128, m, k, 128] m-major flat
    wqk_h = np.ascontiguousarray(
        mt.reshape(KT, P, 16, P).transpose(1, 2, 0, 3).reshape(P, 16 * KT * P)
    ).astype(wdt)

    wv_full = Wv.reshape(C, H * HD)
    wv_h = np.ascontiguousarray(
        wv_full.reshape(KT, P, C).transpose(1, 0, 2).reshape(P, KT * C)
    ).astype(wdt)

    wout_h = np.ascontiguousarray(
        Wout.reshape(KT, P, 8, P).transpose(1, 0, 2, 3).reshape(P, KT * 8 * P)
    ).astype(wdt)

    bqk_h = np.empty((P, 16), dtype=np.float32)
    for m in range(8):
        bqk_h[0:HD, m] = bq[2 * m]
        bqk_h[HD:P, m] = bq[2 * m + 1]
        bqk_h[0:HD, 8 + m] = bk[2 * m]
        bqk_h[HD:P, 8 + m] = bk[2 * m + 1]

    boutp = bout + bv.reshape(H * HD) @ Wout
    bout_h = np.ascontiguousarray(boutp.reshape(8, P).T)

    xb = x.reshape(NB_TOTAL, BS, C)
    in_maps = []
    for core in range(N_CORES):
        blocks = xb[core * NB:(core + 1) * NB]
        xTc = blocks.reshape(TOK, C).T                  # [C, 2048]
        xTt = (xTc.reshape(KT, P, NCH, TCH)
               .transpose(1, 0, 2, 3).reshape(P, KT * NCH * TCH))
        in_maps.append({
            "xT": np.ascontiguousarray(xTt).astype(wdt),
            "wqk": wqk_h, "wv": wv_h, "wout": wout_h,
            "bqk": bqk_h, "bout": bout_h,
        })
    return in_maps


def assemble_output(results):
    """results: list of 8 dicts with 'yT' [128, 8*NCH*TCH] -> full y [B, T, C]."""
    y = np.empty((N_CORES, TOK, C), dtype=np.float32)
    for core, r in enumerate(results):
        yT = r["yT"].reshape(P, 8, NCH, TCH)   # [p, etile, c, i]
        yc = yT.transpose(2, 3, 1, 0).reshape(TOK, C)
        y[core] = yc
    return y.reshape(B, T, C)


_CACHED = {}


def kernel(x, Wqkv, bqkv, Wout, bout):
    from concourse.bass_utils import run_bass_kernel_spmd
    if "nc" not in _CACHED:
        _CACHED["nc"] = _build(reps=1)
    in_maps = prep_inputs(x, Wqkv, bqkv, Wout, bout)
    res = run_bass_kernel_spmd(_CACHED["nc"], in_maps, list(range(N_CORES)))
    return assemble_output(res.results)


# revision 10
# speedup vs baseline: 1.0812x; 1.0812x over previous
"""Batched-normalize softmax pipeline (v3).

Per chunk a (16 head-pairs, TCH=512 tokens):
  S0 (iter a):   QKV projections (PE) -> qk/v tiles
  S1 (iter a+1): per pair: scores mm (PE, hh-major 2-bank psum so the two
                 concurrent tile_position matmuls hit different banks) ->
                 exp (ACT) -> po mm (PE; per-head 128-col v block with a
                 leading ones column puts the softmax denominator in po row
                 0 and v outputs in rows 64-127) -> po_sb copy (ACT, bf16)
                 + denominator reciprocal_approx_fast (DVE, f32, shift-free
                 partition-0 op)
                 per 2 pairs: ACT f32->bf16 convert + ONE gpsimd
                 partition_broadcast (amortizes the Q7 launch; keeps the
                 broadcast off the per-pair critical path)
  S2 (iter a+2): per pair muls (DVE, all-SBUF bf16) -> on tiles
  S3 (iter a+2): out-projection (PE) -> y DMA
PSUM: pex 2x2 banks (scores) + pall 2 (proj/out psums) + ppo 2 (po) = 8.

HW lessons encoded here:
  - per-pair gpsimd ops serialize (~3us launch each on HW vs 95ns in the
    cost model); batch them.
  - nc.vector.reciprocal on a single partition is ~1-2us on HW; use
    reciprocal_approx_fast (f32-only), which also must run WITHOUT a
    partition shift (in/out both partition 0) to produce correct results.
  - SBUF APs must start at partition 0/32/64/96 and may not span more
    than the alignment gap (start 32 -> max 32 partitions).
"""
import numpy as np

import concourse.bass as bass
import concourse.mybir as mybir
import concourse.tile as tile
from concourse import bacc

P = 128
B, T, C = 4, 4096, 1024
H = 16
HD = 64
BS = 256
NB_TOTAL = (B * T) // BS
N_CORES = 8
NB = NB_TOTAL // N_CORES
TOK = NB * BS
KT = C // P
NPAIR = H // 2
TCH = 512
NCH = TOK // TCH

f32 = mybir.dt.float32
bf16 = mybir.dt.bfloat16
W_DT = bf16
ATT_DT = bf16

PAIRS = [(bl, p_) for bl in range(2) for p_ in range(NPAIR)]


def _build(reps: int = 1, variant: str = 'full'):
    nc = bacc.Bacc(None)

    xT = nc.dram_tensor("xT", [P, KT * NCH * TCH], W_DT, kind="ExternalInput")
    wqk = nc.dram_tensor("wqk", [P, 16 * KT * P], W_DT, kind="ExternalInput")
    wv = nc.dram_tensor("wv", [P, KT * C], W_DT, kind="ExternalInput")
    wout = nc.dram_tensor("wout", [P, KT * 8 * P], W_DT, kind="ExternalInput")
    bqk = nc.dram_tensor("bqk", [P, 16], f32, kind="ExternalInput")
    bout = nc.dram_tensor("bout", [P, 8], f32, kind="ExternalInput")
    yT = nc.dram_tensor("yT", [P, 8 * NCH * TCH], f32, kind="ExternalOutput")

    with tile.TileContext(nc) as tc:
        with (
            tc.tile_pool(name="wpool", bufs=1) as wpool,
            tc.tile_pool(name="xpool", bufs=2) as xpool,
            tc.tile_pool(name="qkpool", bufs=32) as qkpool,
            tc.tile_pool(name="vpool", bufs=8) as vpool,
            tc.tile_pool(name="epool", bufs=2) as epool,
            tc.tile_pool(name="popool", bufs=16) as popool,
            tc.tile_pool(name="rcppool", bufs=2) as rcppool,
            tc.tile_pool(name="rcrpool", bufs=8) as rcrpool,
            tc.tile_pool(name="onpool", bufs=8) as onpool,
            tc.tile_pool(name="ypool", bufs=2) as ypool,
            tc.tile_pool(name="pall", bufs=2, space="PSUM") as pall,
            tc.tile_pool(name="pex", bufs=2, space="PSUM") as pex,
            tc.tile_pool(name="ppo", bufs=2, space="PSUM") as ppo,
        ):
            xT_r = xT[:].rearrange("p (k c n) -> p k c n", k=KT, c=NCH)

            bqk_t = wpool.tile([P, 16], f32)
            nc.sync.dma_start(out=bqk_t[:], in_=bqk[:])
            bout_t = wpool.tile([P, 8], f32)
            nc.sync.dma_start(out=bout_t[:], in_=bout[:])
            ones_f = wpool.tile([P, 16], f32)
            nc.vector.memset(ones_f[:], 1.0)
            ones16 = wpool.tile([P, 16], ATT_DT)
            nc.vector.tensor_copy(ones16[:], ones_f[:])
            warm_sb = wpool.tile([P, TCH], ATT_DT)
            nc.vector.memset(warm_sb[:], 0.0)

            def fetch_x(c):
                xt = xpool.tile([P, KT * TCH], W_DT, tag="x")
                nc.sync.dma_start(
                    out=xt[:].rearrange("p (k n) -> p k n", k=KT),
                    in_=xT_r[:, :, c, :])
                return xt

            # single-shot path: the DMA queue drains in emission order, so
            # fetch chunk 0 of x BEFORE the 8MB of weights - projections for
            # chunk 0 then start ~5us in instead of ~30us
            x0_pre = fetch_x(0) if reps == 1 else None

            # one tile per m-slice: qk unit m only waits for its own slice
            wqk_ts = []
            for m in range(16):
                wt = wpool.tile([P, KT * P], W_DT, name=f"wqk_{m}")
                nc.sync.dma_start(
                    out=wt[:], in_=wqk[:, m * KT * P:(m + 1) * KT * P])
                wqk_ts.append(wt)
            wv_t = wpool.tile([P, KT * C], W_DT)
            for k in range(KT):
                nc.sync.dma_start(out=wv_t[:, k * C:(k + 1) * C],
                                  in_=wv[:, k * C:(k + 1) * C])
            wout_t = wpool.tile([P, KT * 8 * P], W_DT)
            for k in range(KT):
                nc.sync.dma_start(out=wout_t[:, k * 8 * P:(k + 1) * 8 * P],
                                  in_=wout[:, k * 8 * P:(k + 1) * 8 * P])

            def emit_qk_unit(xt, m, qk):
                pt = pall.tile([P, TCH], f32, tag="ps")
                for k in range(KT):
                    nc.tensor.matmul(
                        pt[:], wqk_ts[m][:, k * P:(k + 1) * P],
                        xt[:, k * TCH:(k + 1) * TCH],
                        start=(k == 0), stop=(k == KT - 1))
                st = qkpool.tile([P, TCH], ATT_DT, tag="qk")
                nc.scalar.activation(st[:], pt[:],
                                     mybir.ActivationFunctionType.Identity,
                                     bias=bqk_t[:, m:m + 1])
                qk.append(st)

            def emit_v_unit(xt, ts, vt):
                # per-head 128-col block: [ones | 63 pad | 64 v-channels].
                # po then has denom at row 0 (shift-free reciprocal) and v
                # outputs at rows 64-127 (the BIR verifier only allows SBUF
                # accesses spanning 64 partitions from start 0 or 64).
                v_sb = vpool.tile([P, 16 * 128], ATT_DT, tag="v")
                for dch in range(2):
                    pt = pall.tile([P, 512], f32, tag="ps")
                    for k in range(KT):
                        nc.tensor.matmul(
                            pt[:],
                            xt[:, k * TCH + ts * P: k * TCH + (ts + 1) * P],
                            wv_t[:, k * C + dch * 512: k * C + (dch + 1) * 512],
                            start=(k == 0), stop=(k == KT - 1))
                    nc.vector.tensor_copy(
                        v_sb[:, dch * 8 * 128:(dch + 1) * 8 * 128]
                        .rearrange("p (h cc) -> p h cc", h=8)[:, :, 64:128],
                        pt[:].rearrange("p (h cc) -> p h cc", h=8))
                nc.vector.tensor_copy(
                    v_sb[:].rearrange("p (h cc) -> p h cc", h=16)[:, :, 0:1],
                    ones16[:].unsqueeze(2))
                vt.append(v_sb)

            def emit_scores_exp(qk, bl, p_, state):
                # hh-major psum layout: the two concurrent tile_position
                # matmuls (rows 0-63 / 64-127) MUST land in different PSUM
                # banks - same-bank concurrent writes fail on HW.
                co = bl * BS
                qt, kt_ = qk[p_], qk[8 + p_]
                px = pex.tile([P, 4 * BS], f32, tag="pex")
                for jt in range(2):
                    for hh in range(2):
                        lo, hi = hh * HD, (hh + 1) * HD
                        nc.tensor.matmul(
                            px[:, (2 * hh + jt) * BS:(2 * hh + jt + 1) * BS],
                            kt_[lo:hi, co + jt * P: co + (jt + 1) * P],
                            qt[lo:hi, co:co + BS], start=True, stop=True,
                            tile_position=(hh * HD, 0))
                e = epool.tile([P, 4 * BS], ATT_DT, tag="e")
                nc.scalar.activation(
                    e[:], px[:], mybir.ActivationFunctionType.Exp)
                state["e"] = e

            def emit_po(vt, a, i, state, rcp_tiles):
                bl, p_ = PAIRS[i]
                e = state["e"]
                po = ppo.tile([P, 2 * BS], f32, tag="po")
                for hh in range(2):
                    h = 2 * p_ + hh
                    for jt in range(2):
                        nc.tensor.matmul(
                            po[:, hh * BS:(hh + 1) * BS],
                            vt[2 * bl + jt][:, h * 128:(h + 1) * 128],
                            e[:, (2 * hh + jt) * BS:(2 * hh + jt + 1) * BS],
                            start=(jt == 0), stop=(jt == 1))
                po_sb = popool.tile([P, 2 * BS], ATT_DT, tag="posb",
                                    name=f"posb_{a}_{i}")
                nc.scalar.activation(po_sb[:], po[:],
                                     mybir.ActivationFunctionType.Identity)
                if variant != 'dbg_norcp':
                    # denominators are sums of 256 exp() terms of modest
                    # scores (>~1), far from approx_fast's undefined edges
                    eighth, pp = i // 2, i % 2
                    nc.vector.reciprocal_approx_fast(
                        rcp_tiles[eighth][0:1, pp * 2 * BS:(pp + 1) * 2 * BS],
                        po[0:1, :])
                state["posb"] = po_sb

            def emit_bcast(a, quarter, rcp_t):
                rcpb = rcppool.tile([1, 2 * 2 * BS], ATT_DT, tag="rcpb",
                                    name=f"rcpb_{a}_{quarter}")
                nc.scalar.activation(rcpb[:], rcp_t[:],
                                     mybir.ActivationFunctionType.Identity)
                rcr = rcrpool.tile([P, 2 * 2 * BS], ATT_DT, tag="rcr",
                                   name=f"rcr_{a}_{quarter}")
                if variant == 'dbg_nobcast':
                    nc.vector.memset(rcr[:], 0.5)
                elif variant == 'dbg_norcp':
                    nc.vector.memset(rcr[:], 0.5)
                else:
                    nc.gpsimd.partition_broadcast(rcr[:], rcpb[:])
                return rcr

            def emit_muls(i, state, rcr, on_tiles):
                bl, p_ = PAIRS[i]
                co = bl * BS
                pp = i % 2
                po_sb = state["posb"]
                for hh in range(2):
                    nc.vector.tensor_mul(
                        on_tiles[p_][hh * HD:(hh + 1) * HD, co:co + BS],
                        po_sb[64:P, hh * BS:(hh + 1) * BS],
                        rcr[64:P,
                            pp * 2 * BS + hh * BS: pp * 2 * BS + (hh + 1) * BS])

            def emit_out_unit(on_tiles, c, t):
                pt = pall.tile([P, TCH], f32, tag="ps")
                for kk in range(KT):
                    nc.tensor.matmul(
                        pt[:], wout_t[:, (kk * 8 + t) * P:(kk * 8 + t + 1) * P],
                        on_tiles[kk][:], start=(kk == 0), stop=(kk == KT - 1))
                yt = ypool.tile([P, TCH], f32, tag="y")
                nc.scalar.activation(yt[:], pt[:],
                                     mybir.ActivationFunctionType.Identity,
                                     bias=bout_t[:, t:t + 1])
                nc.sync.dma_start(
                    out=yT[:, (t * NCH + c) * TCH:(t * NCH + c + 1) * TCH],
                    in_=yt[:])

            def emit_warmup():
                # the PE p-state ramps 0.65 -> 2.4 GHz only after ~3us of
                # continuous execution, and each For_i iteration's drain
                # resets it; these dummy matmuls ramp the clock during the
                # first x-chunk's DMA, which the PE would spend idle anyway
                wm = pex.tile([P, 4 * BS], f32, tag="pex", name="warm")
                for _ in range(8):
                    nc.tensor.matmul(wm[0:16, 0:TCH], ones16[:], warm_sb[:],
                                     start=True, stop=True)

            def body(x0=None):
                qk, vt, xts = {}, {}, {}
                states, rcps, rcrs, on = {}, {}, {}, {}

                emit_warmup()
                xts[0] = x0 if x0 is not None else fetch_x(0)
                for s in range(NCH + 2):
                    A = s - 1
                    O = s - 2
                    units = []
                    if s < NCH:
                        if s + 1 < NCH:
                            units.append(lambda s=s: xts.__setitem__(
                                s + 1, fetch_x(s + 1)))
                        qk[s] = []
                        vt[s] = []
                        for m in range(16):
                            units.append(
                                lambda m=m, s=s: emit_qk_unit(xts[s], m, qk[s]))
                        for ts in range(4):
                            units.append(
                                lambda ts=ts, s=s: emit_v_unit(xts[s], ts, vt[s]))
                    late_units = []
                    if 0 <= O < NCH:
                        on[O] = [onpool.tile([P, TCH], ATT_DT, tag="on",
                                             name=f"on_{O}_{kk}")
                                 for kk in range(NPAIR)]
                        for i in range(len(PAIRS)):
                            emit_muls(i, states[O][i], rcrs[O][i // 2], on[O])
                        # out units wait on the muls -> bcast(q7) chain that
                        # only resolves a few us into this iteration; emitting
                        # them early head-of-line blocks the in-order PE queue
                        for t in range(8):
                            late_units.append(
                                lambda t=t, O=O: emit_out_unit(on[O], O, t))
                    if 0 <= A < NCH:
                        states[A] = [dict() for _ in PAIRS]
                        rcps[A] = [
                            rcppool.tile([1, 2 * 2 * BS], f32, tag="rcp",
                                         name=f"rcp_{A}_{q}")
                            for q in range(8)]
                        rcrs[A] = [None] * 8
                        ui = 0
                        prev = None
                        np_ = len(PAIRS)
                        half = np_ // 2
                        li = 0
                        for i in range(np_):
                            bl, p_ = PAIRS[i]
                            emit_scores_exp(qk[A], bl, p_, states[A][i])
                            want = (len(units) * (i + 1) + np_ - 1) // np_
                            while ui < min(len(units), want):
                                units[ui]()
                                ui += 1
                            if i >= half:
                                lwant = (len(late_units) * (i - half + 1)
                                         + half - 1) // half
                                while li < min(len(late_units), lwant):
                                    late_units[li]()
                                    li += 1
                            if prev is not None:
                                emit_po(vt[A], A, prev, states[A][prev],
                                        rcps[A])
                                if prev % 2 == 1:
                                    q = prev // 2
                                    rcrs[A][q] = emit_bcast(A, q, rcps[A][q])
                            prev = i
                        emit_po(vt[A], A, prev, states[A][prev], rcps[A])
                        rcrs[A][7] = emit_bcast(A, 7, rcps[A][7])
                        while ui < len(units):
                            units[ui]()
                            ui += 1
                        while li < len(late_units):
                            late_units[li]()
                            li += 1
                    else:
                        for u in units:
                            u()
                        for u in late_units:
                            u()
                    if 0 <= O < NCH:
                        del states[O], rcps[O], rcrs[O], on[O], qk[O], vt[O]
                        if O in xts:
                            del xts[O]

            if reps == 1:
                body(x0_pre)
            else:
                with tc.For_i(0, reps, 1):
                    body()
    nc.finalize()
    return nc


def prep_inputs(x, Wqkv, bqkv, Wout, bout):
    """Host-side shard + repack. Returns list of 8 per-core input dicts."""
    x = np.asarray(x, dtype=np.float32)
    Wqkv = np.asarray(Wqkv, dtype=np.float32)
    bqkv = np.asarray(bqkv, dtype=np.float32)
    Wout = np.asarray(Wout, dtype=np.float32)
    bout = np.asarray(bout, dtype=np.float32)
    wdt = mybir.dt.np(W_DT)

    scale = 1.0 / np.sqrt(HD)
    W3 = Wqkv.reshape(C, H, 3 * HD)
    b3 = bqkv.reshape(H, 3 * HD)
    Wq = W3[:, :, 0:HD] * scale          # [C, H, 64]
    Wk = W3[:, :, HD:2 * HD]
    Wv = W3[:, :, 2 * HD:3 * HD]
    bq = b3[:, 0:HD] * scale
    bk = b3[:, HD:2 * HD]
    bv = b3[:, 2 * HD:3 * HD]

    # m-tiles: m<8 -> [Wq_{2m} | Wq_{2m+1}], m>=8 -> k-pairs
    mt = np.empty((C, 16, P), dtype=np.float32)
    for m in range(8):
        mt[:, m, 0:HD] = Wq[:, 2 * m]
        mt[:, m, HD:P] = Wq[:, 2 * m + 1]
        mt[:, 8 + m, 0:HD] = Wk[:, 2 * m]
        mt[:, 8 + m, HD:P] = Wk[:, 2 * m + 1]
    # -> [128, m, k, 128] m-major flat
    wqk_h = np.ascontiguousarray(
        mt.reshape(KT, P, 16, P).transpose(1, 2, 0, 3).reshape(P, 16 * KT * P)
    ).astype(wdt)

    wv_full = Wv.reshape(C, H * HD)
    wv_h = np.ascontiguousarray(
        wv_full.reshape(KT, P, C).transpose(1, 0, 2).reshape(P, KT * C)
    ).astype(wdt)

    wout_h = np.ascontiguousarray(
        Wout.reshape(KT, P, 8, P).transpose(1, 0, 2, 3).reshape(P, KT * 8 * P)
    ).astype(wdt)

    bqk_h = np.empty((P, 16), dtype=np.float32)
    for m in range(8):
        bqk_h[0:HD, m] = bq[2 * m]
        bqk_h[HD:P, m] = bq[2 * m + 1]
        bqk_h[0:HD, 8 + m] = bk[2 * m]
        bqk_h[HD:P, 8 + m] = bk[2 * m + 1]

    boutp = bout + bv.reshape(H * HD) @ Wout
    bout_h = np.ascontiguousarray(boutp.reshape(8, P).T)

    xb = x.reshape(NB_TOTAL, BS, C)
    in_maps = []
    for core in range(N_CORES):
        blocks = xb[core * NB:(core + 1) * NB]
        xTc = blocks.reshape(TOK, C).T                  # [C, 2048]
        xTt = (xTc.reshape(KT, P, NCH, TCH)
               .transpose(1, 0, 2, 3).reshape(P, KT * NCH * TCH))
        in_maps.append({
            "xT": np.ascontiguousarray(xTt).astype(wdt),
            "wqk": wqk_h, "wv": wv_h, "wout": wout_h,
            "bqk": bqk_h, "bout": bout_h,
        })
    return in_maps


def assemble_output(results):
    """results: list of 8 dicts with 'yT' [128, 8*NCH*TCH] -> full y [B, T, C]."""
    y = np.empty((N_CORES, TOK, C), dtype=np.float32)
    for core, r in enumerate(results):
        yT = r["yT"].reshape(P, 8, NCH, TCH)   # [p, etile, c, i]
        yc = yT.transpose(2, 3, 1, 0).reshape(TOK, C)
        y[core] = yc
    return y.reshape(B, T, C)


_CACHED = {}


def kernel(x, Wqkv, bqkv, Wout, bout):
    from concourse.bass_utils import run_bass_kernel_spmd
    if "nc" not in _CACHED:
        _CACHED["nc"] = _build(reps=1)
    in_maps = prep_inputs(x, Wqkv, bqkv, Wout, bout)
    res = run_bass_kernel_spmd(_CACHED["nc"], in_maps, list(range(N_CORES)))
    return assemble_output(res.results)
